# revision 12
# baseline (speedup 1.0000x reference)
"""v4: two-pass butterfly kernel (low 7 stages + high 3 stages), bf16 output.

Factor B = Bh @ Bl:
  Bl = stages 0..6  — block-diagonal over 8 contiguous 128-position blocks.
  Bh = stages 7..9  — mixes w = pos//128 across the 8 blocks, elementwise in
                      r = pos % 128.

Pass 1 (per 512-batch tile): y^T tiles in "q32" interleaved partition order.
  T[m] (m=0..3 r-range) [128, 2, 512] (h = w-half on the middle axis):
     partition p' = 32*wl + rl  <->  y position (32m + rl) + 128*(4h + wl)
  built by 8 column-packed matmuls (M=32, tile_position) with
  lhsT = Bl^T block slice [128, 32], rhs = x block [128, 512], into a
  2-bank PSUM tile; one [128, 1024] ScalarE eviction per m (bf16).

Pass 2 (per 128-batch chunk): out[b, :] batch-major, bf16.
  For each m-group (256 stored columns = (w_out, rl)):
     psum[:, mi, :] += T[m][h][:, chunk]^T @ D[m][h]   for h = 0, 1
  D[m][h][p', q] = Bh[pos_out, pos_in] (nonzero iff rl_out == rl_in).
  One DVE tensor_add per psum tile (+bias, [128, 2, 8, 32] scatter view)
  writes natural column order; half-tile (2-chunk) stores on the sync queue.
"""

import os
import sys
import numpy as np

for _p in ("/opt/trn_rl_repo", os.path.expanduser("~/.axon_site/_ro/trn_rl_repo")):
    if os.path.isdir(_p) and _p not in sys.path:
        sys.path.insert(0, _p)

import concourse.bass as bass
import concourse.bacc as bacc
import concourse.mybir as mybir
from concourse import tile
from concourse.bass_utils import run_bass_kernel_spmd

import ml_dtypes

N_CORES = 8
BATCH = 32768
N = 1024
LOG_N = 10
BC = BATCH // N_CORES   # 4096 rows per core
BT = 512                # batch tile (pass 1)
NBT = BC // BT          # 8
CHUNKS_PER_BT = BT // 128   # 4

_last_exec_time_ns = None
_nc_cache = None


def _apply_stages(m: np.ndarray, twiddle: np.ndarray, idxs) -> np.ndarray:
    """Apply butterfly stages `idxs` to the rows of m (batch of vectors)."""
    n = N
    for idx in idxs:
        s = 1 << idx
        g = n // (2 * s)
        t = twiddle[0, 0, idx].astype(np.float64).reshape(g, s, 2, 2)
        xr = m.reshape(-1, g, 2, s)
        m = np.einsum("grij,bgjr->bgir", t, xr).reshape(-1, n)
    return m


def _host_weights(twiddle: np.ndarray):
    eye = np.eye(N, dtype=np.float64)
    blt = _apply_stages(eye, twiddle, range(7))        # BlT[k, p] = Bl[p, k]
    bht = _apply_stages(eye, twiddle, range(7, 10))    # BhT[k, p] = Bh[p, k]

    # pass-1 lhsT: bl_pack[k, w, m, r32] = Bl[128w + 32m + r32, 128w + k]
    bl_pack = np.zeros((128, 8, 4, 32), dtype=np.float64)
    for w in range(8):
        blk = blt[128 * w:128 * (w + 1), 128 * w:128 * (w + 1)]  # [k, r]
        bl_pack[:, w] = blk.reshape(128, 4, 32)

    # pass-2 moving operand: d_pack[p', m, h, q]
    #   p' = 32*wl + rl_in  -> pos_in  = 32m + rl_in + 128*(4h + wl)
    #   q  = 32*w_out + rl_out -> pos_out = 32m + rl_out + 128*w_out
    # value = BhT[pos_in, pos_out]
    wl = np.arange(4)[:, None]          # [4, 1]
    rl = np.arange(32)[None, :]         # [1, 32]
    wo = np.arange(8)[:, None]
    d_pack = np.zeros((128, 4, 2, 256), dtype=np.float64)
    for m in range(4):
        for h in range(2):
            pos_in = (32 * m + rl + 128 * (4 * h + wl))        # [4, 32]
            pos_out = (32 * m + rl + 128 * wo)                 # [8, 32]
            # nonzero only when rl_in == rl_out
            sub = bht[np.ix_(pos_in.ravel(), pos_out.ravel())]  # [128, 256]
            mask = (rl.ravel()[None, :].repeat(4, 0).ravel()[:, None]
                    == rl.ravel()[None, :].repeat(8, 0).ravel()[None, :])
            d_pack[:, m, h, :] = np.where(mask, sub, 0.0)

    return bl_pack, d_pack


def _stored_bias(bias: np.ndarray) -> np.ndarray:
    # stored col s = m*256 + w*32 + r  ->  natural pos = 128w + 32m + r
    w = np.arange(8)
    m = np.arange(4)
    r = np.arange(32)
    pos = (128 * w[None, :, None] + 32 * m[:, None, None] + r[None, None, :])
    return np.ascontiguousarray(
        np.broadcast_to(bias[pos.ravel()].astype(np.float32), (128, N))
    )


def _build_nc():
    nc = bacc.Bacc("TRN2", target_bir_lowering=False)
    xtb = nc.dram_tensor("xtb", [NBT, 128, 8, BT], mybir.dt.bfloat16, kind="ExternalInput")
    bl = nc.dram_tensor("bl", [128, 8, 4, 32], mybir.dt.bfloat16, kind="ExternalInput")
    dd = nc.dram_tensor("dd", [128, 4, 2, 256], mybir.dt.bfloat16, kind="ExternalInput")
    bb = nc.dram_tensor("bb", [128, N], mybir.dt.float32, kind="ExternalInput")
    out = nc.dram_tensor("out", [NBT, 128, CHUNKS_PER_BT, N], mybir.dt.bfloat16,
                         kind="ExternalOutput")

    with tile.TileContext(nc) as tc:
        with (
            tc.tile_pool(name="const", bufs=1) as cpool,
            tc.tile_pool(name="tsb", bufs=12) as t_pool,
            tc.tile_pool(name="ot", bufs=3) as ot_pool,
            tc.tile_pool(name="ps1", bufs=6, space="PSUM") as ps1_pool,
            tc.tile_pool(name="ps2", bufs=2, space="PSUM") as ps2_pool,
        ):
            # pass-1 of batch tile 0 is gated only by bls + x tile 0; split
            # the first tile's load so the first matmuls start early
            bls = cpool.tile([128, 8, 4, 32], mybir.dt.bfloat16)
            nc.sync.dma_start(out=bls[:], in_=bl[:])

            xall = cpool.tile([128, 8, BC], mybir.dt.bfloat16)
            nc.sync.dma_start(out=xall[:, 0:2, 0:BT], in_=xtb[0][:, 0:2])
            nc.sync.dma_start(out=xall[:, 2:4, 0:BT], in_=xtb[0][:, 2:4])
            nc.sync.dma_start(out=xall[:, 4:8, 0:BT], in_=xtb[0][:, 4:8])

            dds = cpool.tile([128, 4, 2, 256], mybir.dt.bfloat16)
            nc.sync.dma_start(out=dds[:], in_=dd[:])
            bbt = cpool.tile([128, N], mybir.dt.float32)
            nc.sync.dma_start(out=bbt[:], in_=bb[:])

            # alternate load queues (sync / vector hardware rings) so the
            # early tiles stream at double bandwidth
            for g in range(1, NBT):
                eng = nc.scalar if (g % 2 == 1) else nc.sync
                if g <= 2:
                    eng.dma_start(out=xall[:, 0:4, g * BT:(g + 1) * BT],
                                  in_=xtb[g][:, 0:4])
                    eng.dma_start(out=xall[:, 4:8, g * BT:(g + 1) * BT],
                                  in_=xtb[g][:, 4:8])
                else:
                    eng.dma_start(
                        out=xall[:, :, g * BT:(g + 1) * BT],
                        in_=xtb[g],
                    )

            def pass1(bt):
                bsl = slice(bt * BT, (bt + 1) * BT)
                tsb = {}
                for m in range(4):
                    t_t = t_pool.tile([128, 2, BT], mybir.dt.bfloat16)
                    for h in range(2):
                        ps = ps1_pool.tile([128, BT], mybir.dt.float32)
                        for wl in range(4):
                            w = 4 * h + wl
                            nc.tensor.matmul(
                                ps[32 * wl:32 * (wl + 1), :],
                                bls[:, w, m, :],
                                xall[:, w, bsl],
                                start=True,
                                stop=True,
                                tile_position=(0, 32 * wl),
                            )
                        nc.scalar.copy(out=t_t[:, h, :], in_=ps[:])
                    tsb[m] = t_t
                return tsb

            def pass2(bt, tsb):
                otg = ot_pool.tile([128, CHUNKS_PER_BT, N], mybir.dt.bfloat16)
                for cc in range(CHUNKS_PER_BT):
                    c0 = cc * 128
                    # per-m natural-order view: V[p, m, w, r] = ot[p, 128w+32m+r]
                    ot_v = otg[:, cc, :].rearrange("p (w m r) -> p m w r",
                                                   w=8, m=4, r=32)
                    for half in range(2):
                        ps2 = ps2_pool.tile([128, 2, 256], mybir.dt.float32)
                        for mi in range(2):
                            m = half * 2 + mi
                            for h in range(2):
                                nc.tensor.matmul(
                                    ps2[:, mi, :],
                                    tsb[m][:, h, c0:c0 + 128],
                                    dds[:, m, h, :],
                                    start=(h == 0),
                                    stop=(h == 1),
                                )
                        nc.vector.tensor_add(
                            ot_v[:, 2 * half:2 * half + 2],
                            ps2[:].rearrange("p m (w r) -> p m w r", w=8, r=32),
                            bbt[:, half * 512:(half + 1) * 512].rearrange(
                                "p (m w r) -> p m w r", m=2, w=8, r=32),
                        )
                    # stores ride the scalar engine's hardware DMA queue so
                    # the output stream does not share a ring with the loads
                    if bt == NBT - 1 and cc >= 2:
                        nc.scalar.dma_start(out=out[bt][:, cc:cc + 1],
                                            in_=otg[:, cc:cc + 1])
                    elif cc == 1 or (cc == 3 and bt < NBT - 1):
                        nc.scalar.dma_start(out=out[bt][:, cc - 1:cc + 1],
                                            in_=otg[:, cc - 1:cc + 1])

            # one-tile software pipeline: pass-1 of tile t+1 is emitted before
            # pass-2 of tile t so the PE never waits on the T evictions
            prev = None
            for bt in range(NBT):
                tsb = pass1(bt)
                if prev is not None:
                    pass2(bt - 1, prev)
                prev = tsb
            pass2(NBT - 1, prev)

    nc.compile()
    return nc


def kernel(x: np.ndarray, twiddle: np.ndarray, bias: np.ndarray) -> np.ndarray:
    global _last_exec_time_ns, _nc_cache

    bl_pack, d_pack = _host_weights(twiddle)
    bl_host = np.ascontiguousarray(bl_pack.astype(ml_dtypes.bfloat16))
    d_host = np.ascontiguousarray(d_pack.astype(ml_dtypes.bfloat16))
    bb_host = _stored_bias(np.asarray(bias))

    x = np.ascontiguousarray(x, dtype=np.float32)
    xb = x.astype(ml_dtypes.bfloat16)
    # [cores, NBT, 128 part, 8 w, BT] with tile g contiguous in HBM
    xtb_all = np.ascontiguousarray(
        xb.reshape(N_CORES, NBT, BT, 8, 128).transpose(0, 1, 4, 3, 2)
    )

    if _nc_cache is None:
        _nc_cache = _build_nc()
    nc = _nc_cache

    in_maps = [
        {"xtb": xtb_all[i], "bl": bl_host, "dd": d_host, "bb": bb_host}
        for i in range(N_CORES)
    ]

    trace = bool(int(os.environ.get("BUTTERFLY_TRACE", "0")))
    res = run_bass_kernel_spmd(
        nc,
        in_maps,
        core_ids=list(range(N_CORES)),
        trace=trace,
    )
    _last_exec_time_ns = res.exec_time_ns

    outs = []
    for i in range(N_CORES):
        o = np.asarray(res.results[i]["out"])  # [NBT, 128, 4, N] bf16
        o = o.astype(np.float32).transpose(0, 2, 1, 3).reshape(BC, N)
        outs.append(o)
    return np.concatenate(outs, axis=0)


# revision 15
# speedup vs baseline: 1.0581x; 1.0581x over previous
"""v4: two-pass butterfly kernel (low 7 stages + high 3 stages), bf16 output.

Factor B = Bh @ Bl:
  Bl = stages 0..6  — block-diagonal over 8 contiguous 128-position blocks.
  Bh = stages 7..9  — mixes w = pos//128 across the 8 blocks, elementwise in
                      r = pos % 128.

Pass 1 (per 512-batch tile): y^T tiles in "q32" interleaved partition order.
  T[m] (m=0..3 r-range) [128, 2, 512] (h = w-half on the middle axis):
     partition p' = 32*wl + rl  <->  y position (32m + rl) + 128*(4h + wl)
  built by 8 column-packed matmuls (M=32, tile_position) with
  lhsT = Bl^T block slice [128, 32], rhs = x block [128, 512], into a
  2-bank PSUM tile; one [128, 1024] ScalarE eviction per m (bf16).

Pass 2 (per 128-batch chunk): out[b, :] batch-major, bf16.
  For each m-group (256 stored columns = (w_out, rl)):
     psum[:, mi, :] += T[m][h][:, chunk]^T @ D[m][h]   for h = 0, 1
  D[m][h][p', q] = Bh[pos_out, pos_in] (nonzero iff rl_out == rl_in).
  One DVE tensor_add per psum tile (+bias, [128, 2, 8, 32] scatter view)
  writes natural column order; half-tile (2-chunk) stores on the sync queue.
"""

import os
import sys
import numpy as np

for _p in ("/opt/trn_rl_repo", os.path.expanduser("~/.axon_site/_ro/trn_rl_repo")):
    if os.path.isdir(_p) and _p not in sys.path:
        sys.path.insert(0, _p)

import concourse.bass as bass
import concourse.bacc as bacc
import concourse.mybir as mybir
from concourse import tile
from concourse.bass_utils import run_bass_kernel_spmd

import ml_dtypes

N_CORES = 8
BATCH = 32768
N = 1024
LOG_N = 10
BC = BATCH // N_CORES   # 4096 rows per core
BT = 512                # batch tile (pass 1)
NBT = BC // BT          # 8
CHUNKS_PER_BT = BT // 128   # 4

_last_exec_time_ns = None
_nc_cache = None


def _apply_stages(m: np.ndarray, twiddle: np.ndarray, idxs) -> np.ndarray:
    """Apply butterfly stages `idxs` to the rows of m (batch of vectors)."""
    n = N
    for idx in idxs:
        s = 1 << idx
        g = n // (2 * s)
        t = twiddle[0, 0, idx].astype(np.float64).reshape(g, s, 2, 2)
        xr = m.reshape(-1, g, 2, s)
        m = np.einsum("grij,bgjr->bgir", t, xr).reshape(-1, n)
    return m


def _host_weights(twiddle: np.ndarray):
    eye = np.eye(N, dtype=np.float64)
    blt = _apply_stages(eye, twiddle, range(7))        # BlT[k, p] = Bl[p, k]
    bht = _apply_stages(eye, twiddle, range(7, 10))    # BhT[k, p] = Bh[p, k]

    # pass-1 lhsT: bl_pack[k, w, m, r32] = Bl[128w + 32m + r32, 128w + k]
    bl_pack = np.zeros((128, 8, 4, 32), dtype=np.float64)
    for w in range(8):
        blk = blt[128 * w:128 * (w + 1), 128 * w:128 * (w + 1)]  # [k, r]
        bl_pack[:, w] = blk.reshape(128, 4, 32)

    # pass-2 moving operand: d_pack[p', m, h, q]
    #   p' = 32*wl + rl_in  -> pos_in  = 32m + rl_in + 128*(4h + wl)
    #   q  = 32*w_out + rl_out -> pos_out = 32m + rl_out + 128*w_out
    # value = BhT[pos_in, pos_out]
    wl = np.arange(4)[:, None]          # [4, 1]
    rl = np.arange(32)[None, :]         # [1, 32]
    wo = np.arange(8)[:, None]
    d_pack = np.zeros((128, 4, 2, 256), dtype=np.float64)
    for m in range(4):
        for h in range(2):
            pos_in = (32 * m + rl + 128 * (4 * h + wl))        # [4, 32]
            pos_out = (32 * m + rl + 128 * wo)                 # [8, 32]
            # nonzero only when rl_in == rl_out
            sub = bht[np.ix_(pos_in.ravel(), pos_out.ravel())]  # [128, 256]
            mask = (rl.ravel()[None, :].repeat(4, 0).ravel()[:, None]
                    == rl.ravel()[None, :].repeat(8, 0).ravel()[None, :])
            d_pack[:, m, h, :] = np.where(mask, sub, 0.0)

    return bl_pack, d_pack


def _stored_bias(bias: np.ndarray) -> np.ndarray:
    # stored col s = m*256 + w*32 + r  ->  natural pos = 128w + 32m + r
    w = np.arange(8)
    m = np.arange(4)
    r = np.arange(32)
    pos = (128 * w[None, :, None] + 32 * m[:, None, None] + r[None, None, :])
    return np.ascontiguousarray(
        np.broadcast_to(bias[pos.ravel()].astype(np.float32), (128, N))
    )


def _build_nc():
    nc = bacc.Bacc("TRN2", target_bir_lowering=False)
    xtb = nc.dram_tensor("xtb", [NBT, 128, 8, BT], mybir.dt.bfloat16, kind="ExternalInput")
    bl = nc.dram_tensor("bl", [128, 8, 4, 32], mybir.dt.bfloat16, kind="ExternalInput")
    dd = nc.dram_tensor("dd", [128, 4, 2, 256], mybir.dt.bfloat16, kind="ExternalInput")
    bb = nc.dram_tensor("bb", [128, N], mybir.dt.float32, kind="ExternalInput")
    out = nc.dram_tensor("out", [NBT, 128, CHUNKS_PER_BT, N], mybir.dt.bfloat16,
                         kind="ExternalOutput")

    with tile.TileContext(nc) as tc:
        with (
            tc.tile_pool(name="const", bufs=1) as cpool,
            tc.tile_pool(name="tsb", bufs=12) as t_pool,
            tc.tile_pool(name="ot", bufs=3) as ot_pool,
            tc.tile_pool(name="ps1", bufs=4, space="PSUM") as ps1_pool,
            tc.tile_pool(name="ps2", bufs=4, space="PSUM") as ps2_pool,
        ):
            # pass-1 of batch tile 0 is gated only by bls + x tile 0; split
            # the first tile's load so the first matmuls start early
            bls = cpool.tile([128, 8, 4, 32], mybir.dt.bfloat16)
            nc.sync.dma_start(out=bls[:], in_=bl[:])

            xall = cpool.tile([128, 8, BC], mybir.dt.bfloat16)
            nc.sync.dma_start(out=xall[:, 0:2, 0:BT], in_=xtb[0][:, 0:2])
            nc.sync.dma_start(out=xall[:, 2:4, 0:BT], in_=xtb[0][:, 2:4])
            nc.sync.dma_start(out=xall[:, 4:8, 0:BT], in_=xtb[0][:, 4:8])

            dds = cpool.tile([128, 4, 2, 256], mybir.dt.bfloat16)
            nc.sync.dma_start(out=dds[:], in_=dd[:])
            bbt = cpool.tile([128, N], mybir.dt.float32)
            nc.sync.dma_start(out=bbt[:], in_=bb[:])

            for g in range(1, NBT):
                eng = nc.sync
                if g <= 2:
                    eng.dma_start(out=xall[:, 0:4, g * BT:(g + 1) * BT],
                                  in_=xtb[g][:, 0:4])
                    eng.dma_start(out=xall[:, 4:8, g * BT:(g + 1) * BT],
                                  in_=xtb[g][:, 4:8])
                else:
                    eng.dma_start(
                        out=xall[:, :, g * BT:(g + 1) * BT],
                        in_=xtb[g],
                    )

            def pass1(bt):
                bsl = slice(bt * BT, (bt + 1) * BT)
                tsb = {}
                for m in range(4):
                    t_t = t_pool.tile([128, 2, BT], mybir.dt.bfloat16)
                    for h in range(2):
                        ps = ps1_pool.tile([128, BT], mybir.dt.float32)
                        for wl in range(4):
                            w = 4 * h + wl
                            nc.tensor.matmul(
                                ps[32 * wl:32 * (wl + 1), :],
                                bls[:, w, m, :],
                                xall[:, w, bsl],
                                start=True,
                                stop=True,
                                tile_position=(0, 32 * wl),
                            )
                        nc.scalar.copy(out=t_t[:, h, :], in_=ps[:])
                    tsb[m] = t_t
                return tsb

            def pass2(bt, tsb):
                otg = ot_pool.tile([128, CHUNKS_PER_BT, N], mybir.dt.bfloat16)
                for cc in range(CHUNKS_PER_BT):
                    c0 = cc * 128
                    # per-m natural-order view: V[p, m, w, r] = ot[p, 128w+32m+r]
                    ot_v = otg[:, cc, :].rearrange("p (w m r) -> p m w r",
                                                   w=8, m=4, r=32)
                    for half in range(2):
                        ps2 = ps2_pool.tile([128, 2, 256], mybir.dt.float32)
                        for mi in range(2):
                            m = half * 2 + mi
                            for h in range(2):
                                nc.tensor.matmul(
                                    ps2[:, mi, :],
                                    tsb[m][:, h, c0:c0 + 128],
                                    dds[:, m, h, :],
                                    start=(h == 0),
                                    stop=(h == 1),
                                )
                        nc.vector.tensor_add(
                            ot_v[:, 2 * half:2 * half + 2],
                            ps2[:].rearrange("p m (w r) -> p m w r", w=8, r=32),
                            bbt[:, half * 512:(half + 1) * 512].rearrange(
                                "p (m w r) -> p m w r", m=2, w=8, r=32),
                        )
                    # stores ride the scalar engine's hardware DMA queue so
                    # the output stream does not share a ring with the loads
                    if bt == NBT - 1 and cc >= 2:
                        nc.sync.dma_start(out=out[bt][:, cc:cc + 1],
                                            in_=otg[:, cc:cc + 1])
                    elif cc == 1 or (cc == 3 and bt < NBT - 1):
                        nc.sync.dma_start(out=out[bt][:, cc - 1:cc + 1],
                                            in_=otg[:, cc - 1:cc + 1])

            # one-tile software pipeline: pass-1 of tile t+1 is emitted before
            # pass-2 of tile t so the PE never waits on the T evictions
            prev = None
            for bt in range(NBT):
                tsb = pass1(bt)
                if prev is not None:
                    pass2(bt - 1, prev)
                prev = tsb
            pass2(NBT - 1, prev)

    nc.compile()
    return nc


def kernel(x: np.ndarray, twiddle: np.ndarray, bias: np.ndarray) -> np.ndarray:
    global _last_exec_time_ns, _nc_cache

    bl_pack, d_pack = _host_weights(twiddle)
    bl_host = np.ascontiguousarray(bl_pack.astype(ml_dtypes.bfloat16))
    d_host = np.ascontiguousarray(d_pack.astype(ml_dtypes.bfloat16))
    bb_host = _stored_bias(np.asarray(bias))

    x = np.ascontiguousarray(x, dtype=np.float32)
    xb = x.astype(ml_dtypes.bfloat16)
    # [cores, NBT, 128 part, 8 w, BT] with tile g contiguous in HBM
    xtb_all = np.ascontiguousarray(
        xb.reshape(N_CORES, NBT, BT, 8, 128).transpose(0, 1, 4, 3, 2)
    )

    if _nc_cache is None:
        _nc_cache = _build_nc()
    nc = _nc_cache

    in_maps = [
        {"xtb": xtb_all[i], "bl": bl_host, "dd": d_host, "bb": bb_host}
        for i in range(N_CORES)
    ]

    trace = bool(int(os.environ.get("BUTTERFLY_TRACE", "0")))
    res = run_bass_kernel_spmd(
        nc,
        in_maps,
        core_ids=list(range(N_CORES)),
        trace=trace,
    )
    _last_exec_time_ns = res.exec_time_ns

    outs = []
    for i in range(N_CORES):
        o = np.asarray(res.results[i]["out"])  # [NBT, 128, 4, N] bf16
        o = o.astype(np.float32).transpose(0, 2, 1, 3).reshape(BC, N)
        outs.append(o)
    return np.concatenate(outs, axis=0)
